# revision 18
# baseline (speedup 1.0000x reference)
"""Trainium2 Bass kernel for one batched Kalman-filter update step.

Reference computation (jax):
    x_pred = F @ x                        # [64, 1]
    P_pred = F @ P @ F.T + Q              # [64, 64]
    y      = z - H @ x_pred               # [32, N]
    S      = H @ P_pred @ H.T + R         # [32, 32]
    K      = P_pred @ H.T @ inv(S)        # [64, 32]
    out    = x_pred + K @ y               # [64, N]

All the small-matrix work is O(64^3) and independent of N; the N-scaling
part collapses to `out[:, n] = W @ z[:, n] + c` with W = K [64, 32] and
c = x_pred - K @ H @ x_pred [64, 1].  That is a memory-bound streaming
matmul over N = 1048576 columns: read 128 MB, write 256 MB.

Distribution: pure data parallel.  Each of the 8 NeuronCores handles
131072 columns of z (16 MB in, 32 MB out); the tiny W/c are replicated.
No collectives are needed (forward pass only).

Per-core device pipeline (Tile framework):
  - z shard [32, 131072] viewed as 32 blocks of [32, 4096]; each in-DMA
    brings 4 blocks into a [128, 4096] f32 SBUF tile (2 MB per DMA).
  - Block-diagonal weights Wb [64, 128] = diag(W.T, W.T) stacked twice in
    a [128, 128] SBUF tile so both partition halves have a copy at their
    own base partition.  Matmuls run as fp32r (single-pass fp32) with
    rhs = 64 partitions x 512 cols -> PSUM [128, 512]: output rows 0-63
    are W @ (block j), rows 64-127 are W @ (block j+1).
  - ScalarE evacuates PSUM -> SBUF with a fused per-partition bias add
    (activation Identity, bias = [c; c]).
  - Out-DMA writes [128, 4096] SBUF tiles back as two [64, ...] column
    ranges of the output shard (2 MB per DMA).
"""

import os

import numpy as np

import concourse.bass as bass
import concourse.mybir as mybir
from concourse import bacc
from concourse import tile
from concourse.bass_utils import run_bass_kernel_spmd

N_CORES = 8
STATE_DIM = 64
MEASURE_DIM = 32
N_TOTAL = 1048576
SHARD = N_TOTAL // N_CORES  # 131072 columns per core


def _install_axon_ntff_hook():
    """Provide antenv.axon_hooks so run_bass_kernel_spmd(trace=True) can
    capture NTFF profiles under axon.  The agent image ships a stub antenv
    without axon_hooks; wire the ctypes-based hook from trn_agent_boot to
    the injected libaxon_pjrt.so.  Degrades to hook=None (tracing skipped,
    run still works) on any failure."""
    import sys
    import types

    if "antenv.axon_hooks" in sys.modules:
        return
    hook = None
    try:
        from trn_agent_boot.trn_boot import _ntff_profile_via_ctypes

        so_path = "/opt/axon/libaxon_pjrt.so"
        if os.path.exists(so_path):
            hook = _ntff_profile_via_ctypes(so_path)
    except Exception:
        hook = None
    mod = types.ModuleType("antenv.axon_hooks")
    state = {"hook": hook}
    mod.set_axon_ntff_profile_hook = lambda h: state.__setitem__("hook", h)
    mod.get_axon_ntff_profile_hook = lambda: state["hook"]
    sys.modules["antenv.axon_hooks"] = mod
    try:
        import antenv

        antenv.axon_hooks = mod
    except Exception:
        pass


_install_axon_ntff_hook()

F_BLK = 4096          # columns per block (free dim of one SBUF tile)
G_BLK = 4             # blocks stacked per in-tile (4 * 32 = 128 partitions)
MM_F = 512            # matmul moving free dim (one PSUM bank of fp32)

_CACHE = {}


def _build_nc(shard: int, f_blk: int):
    """Build + compile the per-core Bass program (same program on all cores)."""
    n_blocks = shard // f_blk
    n_tiles = n_blocks // G_BLK
    n_slices = f_blk // MM_F
    f32 = mybir.dt.float32
    f32r = mybir.dt.float32r

    nc = bacc.Bacc(
        "TRN2",
        target_bir_lowering=False,
        debug=False,
        dynamic_dma_scratch_size=32768,
    )

    # z and w are typed float32r end-to-end (same 4 bytes as f32 on the
    # numpy side) so the BIR verifier accepts them as fp32r matmul inputs.
    z = nc.declare_dram_parameter("z", [MEASURE_DIM, shard], f32r, isOutput=False)
    w = nc.declare_dram_parameter("w", [128, 128], f32r, isOutput=False)
    cb = nc.declare_dram_parameter("c", [128, 1], f32, isOutput=False)
    out = nc.declare_dram_parameter("out", [STATE_DIM, shard], f32, isOutput=True)

    # [b, i, f]: block b, z-row i, column f within the block
    Z = z.ap().rearrange("i (b f) -> b i f", f=f_blk)
    # [b, v, f]: block b, out-row v, column f within the block
    O = out.ap().rearrange("v (b f) -> b v f", f=f_blk)

    with tile.TileContext(nc) as tc:
        with (
            tc.tile_pool(name="const", bufs=1) as cpool,
            tc.tile_pool(name="zin", bufs=3) as zpool,
            tc.tile_pool(name="zout", bufs=5) as opool,
            tc.tile_pool(name="ps", bufs=8, space="PSUM") as ppool,
        ):
            wt = cpool.tile([128, 128], f32r)
            nc.sync.dma_start(out=wt[:, :], in_=w.ap()[:, :])
            ct = cpool.tile([128, 1], f32)
            nc.sync.dma_start(out=ct[:, :], in_=cb.ap()[:, :])

            # Routing (silicon-measured): the gpsimd SWDGE ring sprays all
            # 16 SDMA engines (~300 GB/s on pure writes); the SP HWDGE ring
            # gets ~4 engines (~100 GB/s); mixing HBM reads into a ring
            # that also writes degrades its per-engine rate.  ScalarE must
            # not issue DMAs (a waiting DMA at the head of its 8-deep FIFO
            # stalls the PSUM evacuations behind it).  So: outputs all on
            # gpsimd, inputs mostly on sync with some on gpsimd, PSUM
            # evacuation split across Vector and Scalar.
            gp_in = {2, 5, 7}  # 3 of 8 input tiles ride the SWDGE ring
            for t in range(n_tiles):
                zt = zpool.tile([128, f_blk], f32r)
                in_eng = nc.gpsimd if (t % 8) in gp_in else nc.sync
                in_eng.dma_start(out=zt[:, :], in_=Z[4 * t : 4 * t + 4, :, :])
                for h in range(2):
                    ot = opool.tile([128, f_blk], f32)
                    lhsT = wt[64 * h : 64 * h + 64, :]
                    for s in range(n_slices):
                        ps = ppool.tile([128, MM_F], f32)
                        rhs = zt[64 * h : 64 * h + 64, MM_F * s : MM_F * (s + 1)]
                        nc.tensor.matmul(ps[:, :], lhsT, rhs, start=True, stop=True)
                        if h == 0:
                            nc.vector.tensor_scalar_add(
                                ot[:, MM_F * s : MM_F * (s + 1)], ps[:, :], ct[:, :]
                            )
                        else:
                            nc.scalar.add(
                                ot[:, MM_F * s : MM_F * (s + 1)], ps[:, :], ct[:, :]
                            )
                    nc.gpsimd.dma_start(
                        out=O[4 * t + 2 * h : 4 * t + 2 * h + 2, :, :], in_=ot[:, :]
                    )

    nc.compile()
    return nc


def _get_nc():
    key = (SHARD, F_BLK)
    if key not in _CACHE:
        _CACHE[key] = _build_nc(SHARD, F_BLK)
    return _CACHE[key]


def _host_params(F, H, Q, R, P, x):
    """The O(64^3) Kalman small-matrix chain, done once on the host."""
    F = np.asarray(F, dtype=np.float64)
    H = np.asarray(H, dtype=np.float64)
    Q = np.asarray(Q, dtype=np.float64)
    R = np.asarray(R, dtype=np.float64)
    P = np.asarray(P, dtype=np.float64)
    x = np.asarray(x, dtype=np.float64)

    x_pred = F @ x                       # [64, 1]
    P_pred = F @ P @ F.T + Q             # [64, 64]
    S = H @ P_pred @ H.T + R             # [32, 32]
    K = P_pred @ H.T @ np.linalg.inv(S)  # [64, 32]
    c = x_pred - K @ (H @ x_pred)        # [64, 1]

    K32 = K.astype(np.float32)
    # Block-diagonal [64, 128]: out rows 0-63 <- K @ rhs[0:32],
    # rows 64-127 <- K @ rhs[32:64]; stacked twice along partitions.
    Wb = np.zeros((64, 128), dtype=np.float32)
    Wb[0:32, 0:64] = K32.T
    Wb[32:64, 64:128] = K32.T
    W2 = np.concatenate([Wb, Wb], axis=0)          # [128, 128]
    c2 = np.concatenate([c, c], axis=0).astype(np.float32)  # [128, 1]
    return W2, c2


def kernel(z, F, H, Q, R, P, x):
    z = np.asarray(z, dtype=np.float32)
    assert z.shape == (MEASURE_DIM, N_TOTAL), z.shape
    W2, c2 = _host_params(F, H, Q, R, P, x)

    nc = _get_nc()
    in_maps = [
        {
            "z": np.ascontiguousarray(z[:, i * SHARD : (i + 1) * SHARD]),
            "w": W2,
            "c": c2,
        }
        for i in range(N_CORES)
    ]
    res = run_bass_kernel_spmd(nc, in_maps, core_ids=list(range(N_CORES)))
    out = np.concatenate([res.results[i]["out"] for i in range(N_CORES)], axis=1)
    return out
